# revision 1
# baseline (speedup 1.0000x reference)
"""Trainium2 Bass kernel for nn_Block_73443940761664 (moe_routing).

Transformer block: LN1 -> causal MHA -> residual -> LN2 -> top-2-of-8
sparse MoE (dense-equivalent combine) -> residual.

Distribution over 8 NeuronCores:
  dispatch 1: attention head-parallel (2 heads/core) + ReduceScatter of the
              Wo partial sums; LN2/router computed on each core's 256-token
              shard; outputs a, h2 (normalized), router gates.
  host:       top-2 routing -> per-expert token gather (capacity-padded).
  dispatch 2: expert-parallel FFN (1 expert/core) on gathered tokens,
              scaled by gate weight; host scatter-adds into the output.

LayerNorm gains/biases are folded into the consuming weight matrices on the
host (pure weight preprocessing), so the device only computes the
normalization itself. Matmuls run in float32r (fp22 multiply, fp32
accumulate) except the router product, which is true fp32 so that top-2
selection margins survive.
"""

import numpy as np
from contextlib import nullcontext

import concourse.tile as tile
import concourse.mybir as mybir
from concourse import bacc
from concourse.bass_utils import run_bass_kernel_spmd

P = 128
S = 2048
D = 1024
HD = 64           # head dim
HPC = 2           # heads per core
E = 8
F = 4096
C = 768           # per-expert token capacity (padded), multiple of 128
N_CORES = 8
FP = mybir.dt.float32
FPR = mybir.dt.float32r
AF = mybir.ActivationFunctionType
ALU = mybir.AluOpType
EPS = 1e-5

NT = S // P       # 16 token tiles
NG = S // 512     # 4 token groups of 512
ND = D // P       # 8 d-chunks
NF = F // P       # 32 f-tiles
SSH = S // N_CORES  # 256 tokens per core shard


def _layer_norm_tile(nc, eps_ap, x_ap, out_tile, stats_pool):
    """out = (x - mean)/sqrt(var+eps); x [128, D] fp32 sbuf -> out fp32r."""
    st = stats_pool.tile([P, 12], FP, tag="st")
    nc.vector.bn_stats(st[:, 0:6], x_ap[:, 0:512])
    nc.vector.bn_stats(st[:, 6:12], x_ap[:, 512:1024])
    mv = stats_pool.tile([P, 2], FP, tag="mv")
    nc.vector.bn_aggr(mv[:], st[:].rearrange("p (a b) -> p a b", a=2))
    std = stats_pool.tile([P, 1], FP, tag="std")
    nc.scalar.activation(std[:], mv[:, 1:2], AF.Sqrt, bias=eps_ap)
    rstd = stats_pool.tile([P, 1], FP, tag="rstd")
    nc.vector.reciprocal(rstd[:], std[:])
    nmrs = stats_pool.tile([P, 1], FP, tag="nmrs")
    nc.vector.scalar_tensor_tensor(
        out=nmrs[:], in0=mv[:, 0:1], scalar=-1.0, in1=rstd[:],
        op0=ALU.mult, op1=ALU.mult,
    )
    nc.scalar.activation(out_tile[:], x_ap[:], AF.Identity,
                         bias=nmrs[:], scale=rstd[:])
    return rstd, nmrs


def build_stage1(repeat=1, skip_collective=False, skip_attn=False):
    nc = bacc.Bacc("TRN2", target_bir_lowering=False, debug=False,
                   num_devices=N_CORES)
    x = nc.dram_tensor("x", [S, D], FP, kind="ExternalInput").ap()
    wqkv = nc.dram_tensor("wqkv", [D, 3 * P], FPR, kind="ExternalInput").ap()
    bqkv = nc.dram_tensor("bqkv", [1, 3 * P], FPR, kind="ExternalInput").ap()
    wo = nc.dram_tensor("wo", [P, D], FPR, kind="ExternalInput").ap()
    bo8 = nc.dram_tensor("bo8", [1, D], FPR, kind="ExternalInput").ap()
    wr = nc.dram_tensor("wr", [D, E], FP, kind="ExternalInput").ap()
    brr = nc.dram_tensor("brr", [1, E], FP, kind="ExternalInput").ap()
    csw = nc.dram_tensor("csw", [1, E], FP, kind="ExternalInput").ap()
    iden = nc.dram_tensor("iden", [P, P], FPR, kind="ExternalInput").ap()
    onesr = nc.dram_tensor("onesr", [1, 512], FPR, kind="ExternalInput").ap()
    onesc = nc.dram_tensor("onesc", [P, 1], FPR, kind="ExternalInput").ap()
    tmask = nc.dram_tensor("tmask", [4, P, 512], FPR, kind="ExternalInput").ap()
    xsh = nc.dram_tensor("xsh", [SSH, D], FP, kind="ExternalInput").ap()

    a_shard = nc.dram_tensor("a_shard", [SSH, D], FP, kind="ExternalOutput").ap()
    h2_shard = nc.dram_tensor("h2_shard", [SSH, D], FPR, kind="ExternalOutput").ap()
    gates_shard = nc.dram_tensor("gates_shard", [SSH, E], FP,
                                 kind="ExternalOutput").ap()

    a_part = nc.dram_tensor("a_part", [S, D], FP)
    rs_out = nc.dram_tensor("rs_out", [SSH, D], FP)

    with tile.TileContext(nc) as tc:
        with (
            tc.tile_pool(name="xp", bufs=3) as x_pool,
            tc.tile_pool(name="hp", bufs=3) as h_pool,
            tc.tile_pool(name="stats", bufs=4) as stats_pool,
            tc.tile_pool(name="hT", bufs=10) as hT_pool,
            tc.tile_pool(name="qkvT", bufs=1) as qkvT_pool,
            tc.tile_pool(name="vtile", bufs=1) as v_pool,
            tc.tile_pool(name="expT", bufs=4) as exp_pool,
            tc.tile_pool(name="oT", bufs=2) as oT_pool,
            tc.tile_pool(name="misc", bufs=1) as misc_pool,
            tc.tile_pool(name="aout", bufs=3) as a_pool,
            tc.tile_pool(name="small", bufs=2) as small_pool,
            tc.tile_pool(name="ps_tr", bufs=2, space="PSUM") as ps_tr,
            tc.tile_pool(name="ps_mm", bufs=4, space="PSUM") as ps_mm,
            tc.tile_pool(name="ps_acc", bufs=2, space="PSUM") as ps_acc,
        ):
            eps_sb = misc_pool.tile([P, 1], FP)
            nc.vector.memset(eps_sb[:], EPS)
            iden_sb = misc_pool.tile([P, P], FPR)
            nc.sync.dma_start(iden_sb[:], iden[:])
            onesr_sb = misc_pool.tile([1, 512], FPR)
            nc.sync.dma_start(onesr_sb[:], onesr[:])
            onesc_sb = misc_pool.tile([P, 1], FPR)
            nc.sync.dma_start(onesc_sb[:], onesc[:])
            wqkv_sb = []
            for d in range(ND):
                t = misc_pool.tile([P, 3 * P], FPR, tag=f"wqkv{d}")
                nc.sync.dma_start(t[:], wqkv[d * P:(d + 1) * P, :])
                wqkv_sb.append(t)
            bqkv_sb = misc_pool.tile([1, 3 * P], FPR)
            nc.sync.dma_start(bqkv_sb[:], bqkv[:])
            wo_sb = misc_pool.tile([P, D], FPR)
            nc.sync.dma_start(wo_sb[:], wo[:])
            bo8_sb = misc_pool.tile([1, D], FPR)
            nc.sync.dma_start(bo8_sb[:], bo8[:])
            wr_sb = []
            for d in range(ND):
                t = misc_pool.tile([P, E], FP, tag=f"wr{d}")
                nc.sync.dma_start(t[:], wr[d * P:(d + 1) * P, :])
                wr_sb.append(t)
            brr_sb = misc_pool.tile([1, E], FP)
            nc.sync.dma_start(brr_sb[:], brr[:])
            csw_sb = misc_pool.tile([1, E], FP)
            nc.sync.dma_start(csw_sb[:], csw[:])
            brr_bc = misc_pool.tile([P, E], FP)
            nc.gpsimd.partition_broadcast(brr_bc[:], brr_sb[:])
            csw_bc = misc_pool.tile([P, E], FP)
            nc.gpsimd.partition_broadcast(csw_bc[:], csw_sb[:])
            tmask_sb = []
            for j in range(4):
                t = misc_pool.tile([P, 512], FPR, tag=f"tm{j}")
                nc.sync.dma_start(t[:], tmask[j])
                tmask_sb.append(t)

            qT_sb = qkvT_pool.tile([P, S], FPR)   # rows: h0 0:64 | h1 64:128
            kT_sb = qkvT_pool.tile([P, S], FPR)
            vT_sb = qkvT_pool.tile([P, S], FPR)
            v_sb = []
            for kb in range(NT):  # per key tile: h0 [0:65] | h1 [65:130]
                vkb = v_pool.tile([P, HPC * (HD + 1)], FPR, tag=f"v{kb}")
                v_sb.append(vkb)

            for _rep in range(repeat):
                # ---- LN1 + transpose + QKV projections ----
                for g in range(NG):
                    hT = []
                    for _d in range(ND):
                        hTd = hT_pool.tile([P, 512], FPR, tag="hT")
                        hT.append(hTd)
                    for ti in range(4):
                        t_idx = g * 4 + ti
                        xt = x_pool.tile([P, D], FP, tag="x")
                        nc.sync.dma_start(xt[:], x[t_idx * P:(t_idx + 1) * P, :])
                        ht = h_pool.tile([P, D], FPR, tag="h")
                        _layer_norm_tile(nc, eps_sb[:], xt[:], ht, stats_pool)
                        for d in range(ND):
                            ps = ps_tr.tile([P, P], FPR, tag="tr")
                            nc.tensor.transpose(ps[:], ht[:, d * P:(d + 1) * P],
                                                iden_sb[:])
                            nc.vector.tensor_copy(hT[d][:, ti * P:(ti + 1) * P], ps[:])
                    col = slice(g * 512, (g + 1) * 512)
                    for which, dst in ((0, qT_sb), (1, kT_sb), (2, vT_sb)):
                        ps = ps_mm.tile([P, 512], FP, tag="mm")
                        wcol = slice(which * P, (which + 1) * P)
                        for d in range(ND):
                            nc.tensor.matmul(ps[:], wqkv_sb[d][:, wcol], hT[d][:],
                                             start=(d == 0), stop=False)
                        nc.tensor.matmul(ps[:], bqkv_sb[:, wcol], onesr_sb[:],
                                         start=False, stop=True)
                        nc.scalar.activation(dst[:, col], ps[:], AF.Identity)
                    for ti in range(4):
                        kb = g * 4 + ti
                        for h in range(HPC):
                            ps = ps_tr.tile([P, HD], FPR, tag="tr")
                            nc.tensor.transpose(
                                ps[:], vT_sb[h * HD:(h + 1) * HD, kb * P:(kb + 1) * P],
                                iden_sb[h * HD:(h + 1) * HD, h * HD:(h + 1) * HD])
                            base = h * (HD + 1)
                            nc.scalar.activation(v_sb[kb][:, base:base + HD], ps[:],
                                                 AF.Identity)
                            nc.sync.dma_start(
                                v_sb[kb][:, base + HD:base + HD + 1], onesc[:])

                # ---- causal attention + Wo partial ----
                for g in ([] if skip_attn else range(NG)):
                    qcol = slice(g * 512, (g + 1) * 512)
                    oT_sb = oT_pool.tile([P, 512], FPR, tag="oT")
                    for h in range(HPC):
                        acc = ps_acc.tile([HD + 1, 512], FP, tag="acc")
                        nkb = g * 4 + 4
                        for kb in range(nkb):
                            sc = ps_mm.tile([P, 512], FP, tag="mm")
                            nc.tensor.matmul(
                                sc[:], kT_sb[h * HD:(h + 1) * HD, kb * P:(kb + 1) * P],
                                qT_sb[h * HD:(h + 1) * HD, qcol],
                                start=True, stop=True)
                            et = exp_pool.tile([P, 512], FPR, tag="exp")
                            if kb < g * 4:
                                nc.scalar.activation(et[:], sc[:], AF.Exp, scale=0.125)
                            else:
                                j = kb - g * 4
                                etm = exp_pool.tile([P, 512], FPR, tag="exp")
                                nc.scalar.activation(etm[:], sc[:], AF.Exp, scale=0.125)
                                nc.vector.tensor_mul(et[:], etm[:], tmask_sb[j][:])
                            base = h * (HD + 1)
                            nc.tensor.matmul(
                                acc[:], v_sb[kb][:, base:base + HD + 1], et[:],
                                start=(kb == 0), stop=(kb == nkb - 1))
                        den = small_pool.tile([1, 512], FP, tag="den")
                        nc.vector.tensor_copy(den[:], acc[HD:HD + 1, :])
                        rec = small_pool.tile([1, 512], FPR, tag="rec")
                        with nc.allow_low_precision("fp32r softmax denom recip"):
                            nc.vector.reciprocal(rec[:], den[:])
                        bc = ps_mm.tile([HD, 512], FP, tag="mm")
                        nc.tensor.matmul(bc[:], onesr_sb[:, 0:HD], rec[:],
                                         start=True, stop=True)
                        oT_un = exp_pool.tile([HD, 512], FPR, tag="otun")
                        nc.scalar.activation(oT_un[:], acc[0:HD, :], AF.Identity)
                        nc.vector.tensor_mul(oT_sb[h * HD:(h + 1) * HD, :],
                                             oT_un[:], bc[:])
                    for ti in range(4):
                        t_idx = g * 4 + ti
                        asb = a_pool.tile([P, D], FP, tag="a")
                        for half in range(2):
                            colh = slice(half * 512, (half + 1) * 512)
                            ps = ps_mm.tile([P, 512], FP, tag="mm")
                            nc.tensor.matmul(ps[:],
                                             oT_sb[:, ti * P:(ti + 1) * P],
                                             wo_sb[:, colh], start=True, stop=False)
                            nc.tensor.matmul(ps[:], onesr_sb[:, 0:P],
                                             bo8_sb[:, colh], start=False, stop=True)
                            nc.vector.tensor_copy(asb[:, colh], ps[:])
                        nc.sync.dma_start(a_part[t_idx * P:(t_idx + 1) * P, :], asb[:])

                if not skip_collective:
                    nc.gpsimd.collective_compute(
                        "ReduceScatter", ALU.add,
                        replica_groups=[list(range(N_CORES))],
                        ins=[a_part[:]], outs=[rs_out[:]],
                    )

                # ---- post: a = rs + x_slice; LN2; fp32 router; top-2 gates ----
                for tt in range(SSH // P):
                    rst = x_pool.tile([P, D], FP, tag="x")
                    nc.sync.dma_start(rst[:], rs_out[tt * P:(tt + 1) * P, :])
                    xt = x_pool.tile([P, D], FP, tag="x")
                    nc.sync.dma_start(xt[:], xsh[tt * P:(tt + 1) * P, :])
                    at = a_pool.tile([P, D], FP, tag="a")
                    nc.vector.tensor_add(at[:], rst[:], xt[:])
                    nc.sync.dma_start(a_shard[tt * P:(tt + 1) * P, :], at[:])
                    h2t = h_pool.tile([P, D], FPR, tag="h")
                    rstd2, nmrs2 = _layer_norm_tile(nc, eps_sb[:], at[:], h2t,
                                                    stats_pool)
                    nc.sync.dma_start(h2_shard[tt * P:(tt + 1) * P, :], h2t[:])
                    # true-fp32 router product: rawT = Wr_f.T @ a^T
                    lps = ps_mm.tile([E, P], FP, tag="mm")
                    for d in range(ND):
                        ps = ps_tr.tile([P, P], FP, tag="tr")
                        nc.tensor.transpose(ps[:], at[:, d * P:(d + 1) * P],
                                            iden_sb[:].bitcast(FP))
                        aT = hT_pool.tile([P, P], FP, tag="aT")
                        nc.vector.tensor_copy(aT[:], ps[:])
                        nc.tensor.matmul(lps[:], wr_sb[d][:], aT[:],
                                         start=(d == 0), stop=(d == ND - 1))
                    ltr = small_pool.tile([E, P], FP, tag="ltr")
                    nc.scalar.activation(ltr[:], lps[:], AF.Identity)
                    tps = ps_tr.tile([P, E], FP, tag="tr")
                    nc.tensor.transpose(tps[:], ltr[:], iden_sb[0:E, 0:E].bitcast(FP))
                    # token-major LN2 affine fold: logits = rstd*(a@Wr) + nmrs*csw + br
                    ltm = small_pool.tile([P, E], FP, tag="ltmsb")
                    nc.scalar.activation(ltm[:], tps[:], AF.Identity, scale=rstd2[:])
                    nc.vector.scalar_tensor_tensor(
                        out=ltm[:], in0=csw_bc[:], scalar=nmrs2[:], in1=ltm[:],
                        op0=ALU.mult, op1=ALU.add)
                    nc.vector.tensor_add(ltm[:], ltm[:], brr_bc[:])
                    # top-2 softmax gates
                    m1 = small_pool.tile([P, 1], FP, tag="m1")
                    nc.vector.tensor_reduce(m1[:], ltm[:], mybir.AxisListType.X, ALU.max)
                    nm1 = small_pool.tile([P, 1], FP, tag="nm1")
                    nc.vector.tensor_scalar_mul(nm1[:], m1[:], -1.0)
                    ex = small_pool.tile([P, E], FP, tag="ex")
                    nc.scalar.activation(ex[:], ltm[:], AF.Exp, bias=nm1[:])
                    eq = small_pool.tile([P, E], FP, tag="eq")
                    nc.vector.tensor_scalar(out=eq[:], in0=ltm[:], scalar1=m1[:],
                                            scalar2=None, op0=ALU.is_ge)
                    e2 = small_pool.tile([P, E], FP, tag="e2")
                    nc.vector.tensor_mul(e2[:], ex[:], eq[:])
                    nc.vector.tensor_sub(e2[:], ex[:], e2[:])
                    m2 = small_pool.tile([P, 1], FP, tag="m2")
                    nc.vector.tensor_reduce(m2[:], e2[:], mybir.AxisListType.X, ALU.max)
                    msk = small_pool.tile([P, E], FP, tag="msk")
                    nc.vector.tensor_scalar(out=msk[:], in0=ex[:], scalar1=m2[:],
                                            scalar2=None, op0=ALU.is_ge)
                    gp = small_pool.tile([P, E], FP, tag="gp")
                    nc.vector.tensor_mul(gp[:], ex[:], msk[:])
                    dn = small_pool.tile([P, 1], FP, tag="dn")
                    nc.vector.tensor_reduce(dn[:], gp[:], mybir.AxisListType.X, ALU.add)
                    rc = small_pool.tile([P, 1], FP, tag="rc")
                    nc.vector.reciprocal(rc[:], dn[:])
                    gt = small_pool.tile([P, E], FP, tag="gt")
                    nc.scalar.activation(gt[:], gp[:], AF.Identity, scale=rc[:])
                    nc.sync.dma_start(gates_shard[tt * P:(tt + 1) * P, :], gt[:])

    nc.compile()
    return nc


def build_stage2(repeat=1):
    nc = bacc.Bacc("TRN2", target_bir_lowering=False, debug=False,
                   num_devices=N_CORES)
    h2gT = nc.dram_tensor("h2gT", [D, C], FPR, kind="ExternalInput").ap()
    w1 = nc.dram_tensor("w1", [P, NF * ND * P], FPR, kind="ExternalInput").ap()
    b1 = nc.dram_tensor("b1", [F], FP, kind="ExternalInput").ap()
    w2 = nc.dram_tensor("w2", [P, ND * NF * P], FPR, kind="ExternalInput").ap()
    b2 = nc.dram_tensor("b2", [D], FP, kind="ExternalInput").ap()
    gates = nc.dram_tensor("gates", [C], FP, kind="ExternalInput").ap()
    outT = nc.dram_tensor("outT", [D, C], FP, kind="ExternalOutput").ap()

    c_splits = [(0, 512), (512, C - 512)] if C > 512 else [(0, C)]

    with tile.TileContext(nc) as tc:
        with (
            tc.tile_pool(name="h2gT", bufs=ND) as h2gT_pool,
            tc.tile_pool(name="w1p", bufs=2) as w1_pool,
            tc.tile_pool(name="w2p", bufs=2) as w2_pool,
            tc.tile_pool(name="midT", bufs=NF) as midT_pool,
            tc.tile_pool(name="misc", bufs=1) as misc_pool,
            tc.tile_pool(name="outp", bufs=3) as out_pool,
            tc.tile_pool(name="ps_mid", bufs=2, space="PSUM") as ps_mid,
            tc.tile_pool(name="ps_out", bufs=2, space="PSUM") as ps_out,
        ):
            h2gT_sb = []
            for d in range(ND):
                t = h2gT_pool.tile([P, C], FPR, tag="h2gT")
                nc.sync.dma_start(t[:], h2gT[d * P:(d + 1) * P, :])
                h2gT_sb.append(t)
            b1_sb = misc_pool.tile([P, NF], FP)   # b1_sb[p, ft] = b1[ft*128+p]
            nc.sync.dma_start(b1_sb[:], b1.rearrange("(t p) -> p t", p=P))
            b2_sb = misc_pool.tile([P, ND], FP)   # b2_sb[p, dt] = b2[dt*128+p]
            nc.sync.dma_start(b2_sb[:], b2.rearrange("(t p) -> p t", p=P))
            gates_row = misc_pool.tile([1, C], FP)
            nc.sync.dma_start(gates_row[:], gates[None, :])
            gates_bc = misc_pool.tile([P, C], FP)
            nc.gpsimd.partition_broadcast(gates_bc[:], gates_row[:])

            for _rep in range(repeat):
                # phase 1: midT[f, tok] = gelu(w1.T @ h2gT + b1)
                midT_sb = []
                for ft in range(NF):
                    mid_ps = ps_mid.tile([P, C], FP, tag="mid")
                    w1_t = w1_pool.tile([P, ND * P], FPR, tag="w1")
                    nc.sync.dma_start(
                        w1_t[:], w1[:, ft * ND * P:(ft + 1) * ND * P])
                    for (c0, cn) in c_splits:
                        for d in range(ND):
                            nc.tensor.matmul(
                                mid_ps[:, c0:c0 + cn],
                                w1_t[:, d * P:(d + 1) * P],
                                h2gT_sb[d][:, c0:c0 + cn],
                                start=(d == 0),
                                stop=(d == ND - 1),
                            )
                    m = midT_pool.tile([P, C], FPR, tag="midT")
                    nc.scalar.activation(
                        m[:], mid_ps[:], AF.Gelu, bias=b1_sb[:, ft:ft + 1])
                    midT_sb.append(m)

                # phase 2: outT[dcol, tok] = (w2.T @ midT + b2) * gates
                for dt in range(ND):
                    o_ps = ps_out.tile([P, C], FP, tag="out")
                    w2_t = w2_pool.tile([P, NF * P], FPR, tag="w2")
                    for q in range(4):
                        qs = NF * P // 4
                        nc.sync.dma_start(
                            w2_t[:, q * qs:(q + 1) * qs],
                            w2[:, dt * NF * P + q * qs:
                               dt * NF * P + (q + 1) * qs])
                    for ft in range(NF):
                        for (c0, cn) in c_splits:
                            nc.tensor.matmul(
                                o_ps[:, c0:c0 + cn],
                                w2_t[:, ft * P:(ft + 1) * P],
                                midT_sb[ft][:, c0:c0 + cn],
                                start=(ft == 0), stop=(ft == NF - 1))
                    o_sb = out_pool.tile([P, C], FP, tag="osb")
                    nc.vector.scalar_tensor_tensor(
                        out=o_sb[:], in0=o_ps[:], scalar=b2_sb[:, dt:dt + 1],
                        in1=gates_bc[:], op0=ALU.add, op1=ALU.mult)
                    nc.sync.dma_start(outT[dt * P:(dt + 1) * P, :], o_sb[:])

    nc.compile()
    return nc


_CACHE = {}


def _get_stage(name, repeat=1, **kw):
    key = (name, repeat, tuple(sorted(kw.items())))
    if key not in _CACHE:
        nc = (build_stage1(repeat, **kw) if name == "s1"
              else build_stage2(repeat, **kw))
        _CACHE[key] = _make_runner(nc)
    return _CACHE[key]


def _make_runner(nc):
    """Build a reusable sharded jitted callable for an SPMD bass program."""
    import jax
    from jax.sharding import Mesh, PartitionSpec
    from jax.experimental.shard_map import shard_map
    import concourse.bass2jax as bass2jax

    bass2jax.install_neuronx_cc_hook()
    partition_name = nc.partition_id_tensor.name if nc.partition_id_tensor else None
    in_names, out_names, out_avals, zero_outs = [], [], [], []
    for alloc in nc.m.functions[0].allocations:
        if not isinstance(alloc, mybir.MemoryLocationSet):
            continue
        name = alloc.memorylocations[0].name
        if alloc.kind == "ExternalInput":
            if name != partition_name:
                in_names.append(name)
        elif alloc.kind == "ExternalOutput":
            out_names.append(name)
            shape = tuple(alloc.tensor_shape)
            dtype = mybir.dt.np(alloc.dtype)
            out_avals.append(jax.core.ShapedArray(shape, dtype))
            zero_outs.append(np.zeros(shape, dtype))
    n_params = len(in_names)
    n_outs = len(out_avals)
    in_names_all = in_names + out_names
    if partition_name is not None:
        in_names_all = in_names_all + [partition_name]

    def _body(*args):
        operands = list(args)
        if partition_name is not None:
            operands.append(bass2jax.partition_id_tensor())
        outs = bass2jax._bass_exec_p.bind(
            *operands,
            out_avals=tuple(out_avals),
            in_names=tuple(in_names_all),
            out_names=tuple(out_names),
            lowering_input_output_aliases=(),
            sim_require_finite=True,
            sim_require_nnan=True,
            nc=nc,
        )
        return tuple(outs)

    devices = jax.devices()[:N_CORES]
    mesh = Mesh(np.asarray(devices), ("core",))
    in_specs = (PartitionSpec("core"),) * (n_params + n_outs)
    out_specs = (PartitionSpec("core"),) * len(out_names)
    sharded = jax.jit(
        shard_map(_body, mesh=mesh, in_specs=in_specs, out_specs=out_specs,
                  check_rep=False),
        keep_unused=True,
    )

    class Runner:
        pass

    r = Runner()
    r.nc = nc
    r.sharded = sharded
    r.in_names = in_names
    r.out_names = out_names
    r.zero_outs = zero_outs
    r.out_avals = out_avals
    return r


def _run_spmd(runner, in_maps):
    concat_in = [
        np.concatenate([np.asarray(in_maps[c][nm]) for c in range(N_CORES)],
                       axis=0)
        for nm in runner.in_names
    ]
    concat_zeros = [
        np.zeros((N_CORES * z.shape[0], *z.shape[1:]), z.dtype)
        for z in runner.zero_outs
    ]
    outs = runner.sharded(*concat_in, *concat_zeros)
    return [
        {nm: np.asarray(outs[i]).reshape(N_CORES, *runner.out_avals[i].shape)[c]
         for i, nm in enumerate(runner.out_names)}
        for c in range(N_CORES)
    ]


def _stage1_in_maps(inputs):
    x = np.ascontiguousarray(np.asarray(inputs["x"], np.float32)[0])
    g1 = np.asarray(inputs["ln1_g"], np.float32)
    b1v = np.asarray(inputs["ln1_b"], np.float32)
    g2 = np.asarray(inputs["ln2_g"], np.float32)
    b2v = np.asarray(inputs["ln2_b"], np.float32)
    Wq, bq = np.asarray(inputs["Wq"], np.float32), np.asarray(inputs["bq"], np.float32)
    Wk, bk = np.asarray(inputs["Wk"], np.float32), np.asarray(inputs["bk"], np.float32)
    Wv, bv = np.asarray(inputs["Wv"], np.float32), np.asarray(inputs["bv"], np.float32)
    Wo, bo = np.asarray(inputs["Wo"], np.float32), np.asarray(inputs["bo"], np.float32)
    Wr, br = np.asarray(inputs["Wr"], np.float32), np.asarray(inputs["br"], np.float32)

    Wqf, bqf = g1[:, None] * Wq, bq + b1v @ Wq
    Wkf, bkf = g1[:, None] * Wk, bk + b1v @ Wk
    Wvf, bvf = g1[:, None] * Wv, bv + b1v @ Wv
    Wrf, brf = g2[:, None] * Wr, br + b2v @ Wr

    tri = np.triu(np.ones((P, P), np.float32))
    tmask = np.zeros((4, P, 512), np.float32)
    for j in range(4):
        for m in range(4):
            blk = (np.ones((P, P), np.float32) if m > j
                   else tri if m == j else np.zeros((P, P), np.float32))
            tmask[j][:, m * P:(m + 1) * P] = blk

    common = dict(
        x=x,
        iden=np.eye(P, dtype=np.float32),
        onesr=np.ones((1, 512), np.float32),
        onesc=np.ones((P, 1), np.float32),
        tmask=tmask,
        wr=np.ascontiguousarray(Wrf.astype(np.float32)),
        brr=brf.astype(np.float32)[None, :],
        csw=Wrf.sum(axis=0).astype(np.float32)[None, :],
        bo8=(bo / 8.0).astype(np.float32)[None, :],
    )
    in_maps = []
    for c in range(N_CORES):
        cols = slice(c * HPC * HD, (c + 1) * HPC * HD)
        wqkv = np.concatenate([Wqf[:, cols], Wkf[:, cols], Wvf[:, cols]],
                              axis=1).astype(np.float32)
        bqkv = np.concatenate([bqf[cols], bkf[cols], bvf[cols]]).astype(
            np.float32)[None, :]
        m = dict(common)
        m.update(
            wqkv=np.ascontiguousarray(wqkv),
            bqkv=bqkv,
            wo=np.ascontiguousarray(Wo[cols, :].astype(np.float32)),
            xsh=np.ascontiguousarray(x[c * SSH:(c + 1) * SSH]),
        )
        in_maps.append({k: np.ascontiguousarray(v, dtype=np.float32)
                        for k, v in m.items()})
    return in_maps


def kernel(**inputs):
    r1 = _get_stage("s1")
    in_maps1 = _stage1_in_maps(inputs)
    res1 = _run_spmd(r1, in_maps1)

    a = np.concatenate([res1[c]["a_shard"] for c in range(N_CORES)])
    h2 = np.concatenate([res1[c]["h2_shard"] for c in range(N_CORES)])
    gates = np.concatenate([res1[c]["gates_shard"] for c in range(N_CORES)])

    g2 = np.asarray(inputs["ln2_g"], np.float32)
    b2v = np.asarray(inputs["ln2_b"], np.float32)
    e_w1 = np.asarray(inputs["e_w1"], np.float32)
    e_b1 = np.asarray(inputs["e_b1"], np.float32)
    e_w2 = np.asarray(inputs["e_w2"], np.float32)
    e_b2 = np.asarray(inputs["e_b2"], np.float32)

    r2 = _get_stage("s2")
    in_maps2 = []
    idxs = []
    for e in range(N_CORES):
        idx = np.nonzero(gates[:, e] > 0.0)[0]
        assert len(idx) <= C, f"expert {e} overflow: {len(idx)} > {C}"
        idxs.append(idx)
        h2g = np.zeros((C, D), np.float32)
        h2g[:len(idx)] = h2[idx]
        gv = np.zeros((C,), np.float32)
        gv[:len(idx)] = gates[idx, e]
        w1f = (g2[:, None] * e_w1[e]).astype(np.float32)
        b1f = e_b1[e] + b2v @ e_w1[e]
        w1host = np.ascontiguousarray(
            w1f.reshape(ND, P, NF, P).transpose(1, 2, 0, 3).reshape(
                P, NF * ND * P))
        w2host = np.ascontiguousarray(
            e_w2[e].reshape(NF, P, ND, P).transpose(1, 2, 0, 3).reshape(
                P, ND * NF * P))
        in_maps2.append(dict(
            h2gT=np.ascontiguousarray(h2g.T),
            w1=w1host,
            b1=b1f.astype(np.float32),
            w2=w2host,
            b2=e_b2[e],
            gates=gv,
        ))
    res2 = _run_spmd(r2, in_maps2)

    out = a.copy()
    for e in range(N_CORES):
        idx = idxs[e]
        out[idx] += res2[e]["outT"][:, :len(idx)].T
    return out.reshape(1, S, D).astype(np.float32)

